# revision 1
# baseline (speedup 1.0000x reference)
"""GenerativeInfoNCE loss on 8 Trainium2 NeuronCores (Bass/Tile).

Strategy (data-parallel over batch, per the sharding hint):
  - Shard the 32 batches across 8 cores (4 batches / core -> 2044 rows of
    (b, s) prediction positions, padded to 2048 = 16 tiles of 128).
  - Every core gets the FULL event table (16384 x 1024, bf16) in its HBM;
    negatives are fetched with indirect-DMA row gathers (128 rows x 10
    negatives per tile in one descriptor batch).
  - Per 128-row tile: fused multiply+reduce (tensor_tensor_reduce) on the
    Vector engine produces the 10 negative logits and the positive-score
    ingredients (dot, |p|^2, |q|^2); Scalar engine does sqrt/exp/ln.
  - logsumexp over the 11 logits is batched once at the end for all 16
    tiles; per-row losses are DMA'd out and the host does the final mean
    in float64 (the "psum-mean" step of the hint, done on host since it
    is 16K scalars).

The index remap (skip own batch's block of S events) is pure int math and
is done on the host in numpy before sharding.
"""

import numpy as np

import concourse.bacc as bacc
import concourse.bass as bass
import concourse.tile as tile
from concourse import mybir
from concourse.bass_utils import run_bass_kernel_spmd

B, S, H, NEG = 32, 512, 1024, 10
NCORES = 8
BPC = B // NCORES            # batches per core
R = BPC * (S - 1)            # valid rows per core (2044)
NT = (R + 127) // 128        # 16 tiles of 128 rows
RP = NT * 128                # padded rows (2048)
K = NEG + 1                  # logits per row

BF16 = mybir.dt.bfloat16
F32 = mybir.dt.float32
I32 = mybir.dt.int32

NQ = 4  # SWDGE queues to spread gather descriptor generation across


def _indirect_gather(eng, out, in_, offset_ap, queue_num: int):
    """bass.BassGpSimd.indirect_dma_start (gather direction only), with a
    selectable SWDGE queue instead of the hardcoded qPoolDynamic."""
    assert in_.space == bass.MemorySpace.DRAM
    assert out.space == bass.MemorySpace.SBUF
    assert isinstance(in_.offset, int) and in_.offset == 0
    out_ap = eng.lower_ap_dma(out, for_indirect_dma=True)
    in_ap = eng.lower_ap_dma(in_, for_indirect_dma=True)
    assert len(in_ap) == 1 and len(out_ap) == 1
    off = eng.lower_ap_dma(offset_ap)
    assert len(off) == 1
    in_ap.append(off[0])
    ap_shape = in_.shape
    coef = 1
    for i in range(1, len(ap_shape)):
        coef *= ap_shape[i]
    in_ap[0].dynamic_ap_info = mybir.DynamicAccessPatternInfo(
        c=0,
        actual_ap=out.ap,
        indirect_dim_max_index=ap_shape[0],
        offset_expr=[
            mybir.DynamicAccessPatternOffsetExpr(
                coef=coef,
                aff_expr=mybir.DynamicAccessPatternOffsetExprAffExpr(
                    kind="IndirectArgId", arg_id=1),
            )
        ],
    )
    return eng.add_instruction(
        mybir.InstDMACopy(
            name=eng.bass.get_next_instruction_name(),
            queue=f"qPoolDynamic{queue_num or ''}",
            mode="Copy",
            ins=in_ap,
            outs=out_ap,
            oob_is_err=True,
            cce_op=mybir.AluOpType.bypass,
        )
    )


def _build(temp: float, reps: int = 1, no_gather: bool = False,
           no_dot: bool = False):
    """Build + compile the per-core program (identical on all 8 cores).

    reps > 1 wraps the whole body in a hardware For loop (used only for
    timing; the work is identical every iteration). no_gather / no_dot
    ablate the negative-gather DMAs / the DVE dot products (timing
    experiments only — results are wrong with either set).
    """
    nc = bacc.Bacc("TRN2", target_bir_lowering=False, debug=False,
                   num_devices=NCORES)

    ev_d = nc.dram_tensor("events", [B * S, H], BF16, kind="ExternalInput")
    pred_d = nc.dram_tensor("pred", [RP, H], BF16, kind="ExternalInput")
    pos_d = nc.dram_tensor("pos", [RP, H], BF16, kind="ExternalInput")
    idx_d = nc.dram_tensor("idx", [128, NT * NEG], I32, kind="ExternalInput")
    out_d = nc.dram_tensor("loss", [128, NT], F32, kind="ExternalOutput")

    inv_t = 1.0 / temp
    mult = mybir.AluOpType.mult
    add = mybir.AluOpType.add
    AF = mybir.ActivationFunctionType
    X = mybir.AxisListType.X

    with tile.TileContext(nc) as tc:
        import contextlib
        with contextlib.ExitStack() as ctx:
            io = ctx.enter_context(tc.tile_pool(name="io", bufs=4))
            gp = ctx.enter_context(tc.tile_pool(name="gather", bufs=24))
            scrp = ctx.enter_context(tc.tile_pool(name="scratch", bufs=2))
            sm = ctx.enter_context(tc.tile_pool(name="small", bufs=8))
            pers = ctx.enter_context(tc.tile_pool(name="persist", bufs=1))

            loop_cm = tc.For_i(0, reps, 1) if reps > 1 else None
            if loop_cm is not None:
                ctx.enter_context(loop_cm)

            idx_t = pers.tile([128, NT * NEG], I32, tag="idx")
            nc.sync.dma_start(out=idx_t[:], in_=idx_d.ap())
            logits = pers.tile([128, NT * K], F32, tag="logits")

            for t in range(NT):
                rs = slice(t * 128, (t + 1) * 128)
                pred_t = io.tile([128, H], BF16, tag="pred")
                nc.sync.dma_start(out=pred_t[:], in_=pred_d.ap()[rs, :])
                pos_t = io.tile([128, H], BF16, tag="pos")
                nc.sync.dma_start(out=pos_t[:], in_=pos_d.ap()[rs, :])

                # NOTE: indirect DMA on real HW only supports one index per
                # partition ([128,1] offset AP) — CoreSim accepts [128,NEG]
                # but the descriptors walk the wrong addresses. One gather
                # per negative.
                # One small SBUF tile per negative so each dot only waits
                # for its own gather (a shared [128,NEG,H] tile would make
                # the first dot RAW-wait on all NEG gathers).
                # scalar_tensor_tensor: out=(in0*scalar) op1 in1 with a
                # fused fp32 row-sum into accum_out. (tensor_tensor_reduce
                # is NOT usable: it hard-crashes the device on this HW/NEFF
                # path, in both f32 and bf16.)
                scr = scrp.tile([128, H], BF16, tag="scr")
                for j in range(NEG):
                    gj = gp.tile([128, H], BF16, tag="g")
                    if not no_gather:
                        nc.gpsimd.indirect_dma_start(
                            out=gj[:],
                            out_offset=None,
                            in_=ev_d.ap(),
                            in_offset=bass.IndirectOffsetOnAxis(
                                ap=idx_t[:, t * NEG + j:t * NEG + j + 1],
                                axis=0),
                        )
                    if not no_dot:
                        c = t * K + 1 + j
                        nc.vector.scalar_tensor_tensor(
                            out=scr[:], in0=pred_t[:], scalar=inv_t,
                            in1=gj[:], op0=mult, op1=mult,
                            accum_out=logits[:, c:c + 1],
                        )
                if no_dot:
                    nc.vector.memset(logits[:, t * K + 1:(t + 1) * K], 1.0)

                pn2 = sm.tile([128, 1], F32, tag="pn2")
                qn2 = sm.tile([128, 1], F32, tag="qn2")
                ppd = sm.tile([128, 1], F32, tag="ppd")
                nc.vector.scalar_tensor_tensor(
                    out=scr[:], in0=pred_t[:], scalar=1.0, in1=pred_t[:],
                    op0=mult, op1=mult, accum_out=pn2[:])
                nc.vector.scalar_tensor_tensor(
                    out=scr[:], in0=pos_t[:], scalar=1.0, in1=pos_t[:],
                    op0=mult, op1=mult, accum_out=qn2[:])
                nc.vector.scalar_tensor_tensor(
                    out=scr[:], in0=pred_t[:], scalar=inv_t, in1=pos_t[:],
                    op0=mult, op1=mult, accum_out=ppd[:])
                nrm = sm.tile([128, 1], F32, tag="nrm")
                nc.vector.tensor_mul(out=nrm[:], in0=pn2[:], in1=qn2[:])
                nc.scalar.activation(out=nrm[:], in_=nrm[:], func=AF.Sqrt)
                rn = sm.tile([128, 1], F32, tag="rn")
                nc.vector.reciprocal(out=rn[:], in_=nrm[:])
                nc.vector.tensor_mul(
                    out=logits[:, t * K:t * K + 1], in0=ppd[:], in1=rn[:])

            # Batched logsumexp over all NT tiles at once.
            l3 = logits[:].rearrange("p (t k) -> p t k", k=K)
            m = pers.tile([128, NT, 1], F32, tag="m")
            nc.vector.reduce_max(out=m[:], in_=l3, axis=X)
            sh = pers.tile([128, NT, K], F32, tag="sh")
            nc.vector.tensor_sub(out=sh[:], in0=l3, in1=m[:].to_broadcast([128, NT, K]))
            eh = pers.tile([128, NT, K], F32, tag="eh")
            nc.scalar.activation(out=eh[:], in_=sh[:], func=AF.Exp)
            ss = pers.tile([128, NT, 1], F32, tag="ss")
            nc.vector.reduce_sum(out=ss[:], in_=eh[:], axis=X)
            nc.scalar.activation(out=ss[:], in_=ss[:], func=AF.Ln)
            outt = pers.tile([128, NT], F32, tag="outt")
            nc.vector.tensor_add(out=outt[:], in0=m[:, :, 0], in1=ss[:, :, 0])
            nc.vector.tensor_sub(out=outt[:], in0=outt[:], in1=l3[:, :, 0])
            nc.sync.dma_start(out=out_d.ap(), in_=outt[:])

    nc.compile()
    return nc


def _prep_in_maps(encoder_outputs, event_embeddings, neg_indices):
    enc = np.asarray(encoder_outputs, dtype=np.float32)
    ev = np.asarray(event_embeddings, dtype=np.float32)
    ni = np.asarray(neg_indices)
    bf = mybir.dt.np(BF16)

    b_ids = np.arange(B, dtype=ni.dtype)[:, None, None]
    gidx = (ni + S * (ni >= b_ids * S).astype(ni.dtype)).astype(np.int32)

    ev_flat = np.ascontiguousarray(ev.reshape(B * S, H)).astype(bf)

    in_maps = []
    for c in range(NCORES):
        bs = slice(c * BPC, (c + 1) * BPC)
        pred = enc[bs, :-1, :].reshape(R, H)
        pos = ev[bs, 1:, :].reshape(R, H)
        pred_p = np.ones((RP, H), np.float32)
        pred_p[:R] = pred
        pos_p = np.ones((RP, H), np.float32)
        pos_p[:R] = pos
        idx = np.zeros((RP, NEG), np.int32)
        idx[:R] = gidx[bs].reshape(R, NEG)
        # device layout: [partition, tile, neg]
        idx_pt = np.ascontiguousarray(
            idx.reshape(NT, 128, NEG).transpose(1, 0, 2)).reshape(128, NT * NEG)
        in_maps.append({
            "events": ev_flat,
            "pred": pred_p.astype(bf),
            "pos": pos_p.astype(bf),
            "idx": idx_pt,
        })
    return in_maps


def _reduce_loss(results) -> np.float32:
    total = 0.0
    for c in range(NCORES):
        lr = np.asarray(results[c]["loss"], dtype=np.float64)  # [128, NT]
        rows = lr.T.reshape(RP)[:R]
        total += rows.sum()
    return np.float32(total / (B * (S - 1)))


_NC_CACHE: dict = {}


def kernel(encoder_outputs, event_embeddings, neg_indices, temperature):
    temp = float(np.asarray(temperature))
    nc = _NC_CACHE.get(temp)
    if nc is None:
        nc = _build(temp)
        _NC_CACHE[temp] = nc
    in_maps = _prep_in_maps(encoder_outputs, event_embeddings, neg_indices)
    res = run_bass_kernel_spmd(nc, in_maps, core_ids=list(range(NCORES)))
    return _reduce_loss(res.results)

